# revision 38
# baseline (speedup 1.0000x reference)
"""Trainium2 Bass kernel for the (non-standard) MultiHeadAttention module.

Reference math (B=4, N=2048, E=512, H=8):
    q/k/v  = x @ W{q,k,v} + b          # (B, N, E*H)
    split:   head h takes columns h::H  -> per-head (N, E) matrices
    attT_h = (k_h^T @ q_h) * 1/sqrt(N) # (f, e) -- attention over the E axis
    A_h    = exp(attT_h)               # softmax numerator (no max-sub
                                       #  needed, logits are O(+-5))
    s_h[e] = sum_f A_h[f, e]
    out row n' = 4e + r gets  sum_hl (A_h^T/s_h) @ P_h + bp
      for h = 2r + hl  (consequence of the reference's raw
      (B,E,H,N)->(B,N,E*H) reshape before the output projection), where
    P_h    = v_h^T @ Wp_half(hl) + bp/2

Key algebraic refactors (this module attends over the E axis and contracts
over n, so everything collapses into E x E space):
  * Gram matrix  X = x_b^T @ x_b  (E x E, once per core; only the upper
    block-triangle is computed, the rest comes from PE transposes):
      attT_h = Wk_h^T X Wq_h + (Wk_h^T xs) (x) bq_h
               + bk_h (x) (Wq_h^T xs + N bq_h),   xs = colsum(x_b)
    -- eliminates the q/k projections entirely.
  * (A @ v^T) @ Wp == A @ (v^T @ Wp) and
    v_h^T @ Wp_hl == Wv_h^T @ G_hl + bv_h (x) colsum(Wp_hl)  with
    G_hl = x_b^T @ Wp_hl computed once per core -- eliminates the v
    projection and the big P matmuls.
  * bp/2 folded into each P_h; softmax normalization at the very end:
    out = U0*r0 + U1*r1,  U_h = A_h^T @ P_h,  r_h = 1/s_h.
  * s_h computed with ones as the stationary operand (A moving, 4 wide
    matmuls) then transposed into e-partition layout -- avoids 16
    LDWEIGHTS-bound tiny matmuls per head.
  * all rank-1 bias updates are batched into single contraction-2 matmuls.

Everything runs in bf16 (inputs cast host-side; fp32 PSUM accumulate),
which keeps the PE at full speed and halves DMA + SBUF.

Sharding: 16 independent units (b, r), b in 0..3, r in 0..3; unit (b, r)
owns heads {2r, 2r+1} and produces output rows out[b, r::4, :].  Two units
per core, batch-major:  core c -> b = c//2, r in {2*(c%2), 2*(c%2)+1}.
No inter-core communication.
"""

import ml_dtypes
import numpy as np
from contextlib import ExitStack

import concourse.bass as bass
import concourse.mybir as mybir
import concourse.tile as tile
from concourse import bacc
from concourse.bass_utils import run_bass_kernel_spmd

BF16_NP = ml_dtypes.bfloat16

B, N, E, H = 4, 2048, 512, 8
NT = N // 128          # 16 contraction chunks of 128 over n
EB = E // 128          # 4 blocks of 128 over e/f
SCALE = float(1.0 / np.sqrt(np.float32(N)))
F32 = mybir.dt.float32
BF16 = mybir.dt.bfloat16
PSUM = bass.MemorySpace.PSUM

_CACHED_NC = None


def build_nc():
    nc = bacc.Bacc("TRN2", target_bir_lowering=False, debug=False)

    xn_d = nc.dram_tensor("xn", (N, E), BF16, kind="ExternalInput")
    wq_d = nc.dram_tensor("wq", (2, 2, 128, EB, E), BF16, kind="ExternalInput")
    wk_d = nc.dram_tensor("wk", (2, 2, 128, EB, E), BF16, kind="ExternalInput")
    wv_d = nc.dram_tensor("wv", (2, 2, 128, EB, E), BF16, kind="ExternalInput")
    # wp holds only this core's n-half of Wp (G is contracted over half of
    # n per core and pair-AllReduced)
    wp_d = nc.dram_tensor("wp", (2, N // 2, E), BF16, kind="ExternalInput")
    # biasx[p, u, hl, which, :]:
    #   which 0: p0 = Wk_h^T xs, p1 = bk_h          (attT stationary)
    #   which 1: p0 = bq_h, p1 = Wq_h^T xs + N bq_h (attT moving)
    #   which 2: p0 = bv_h, p1 = 1.0                (P stationary)
    #   which 3: p0 = colsum(Wp_hl), p1 = bp/2      (P moving)
    biasx_d = nc.dram_tensor("biasx", (2, 2, 2, 4, E), BF16, kind="ExternalInput")
    eye_d = nc.dram_tensor("eye", (128, 128), BF16, kind="ExternalInput")
    onescol_d = nc.dram_tensor("onescol", (128, 1), BF16, kind="ExternalInput")
    out_d = nc.dram_tensor("out", (2, E, E), BF16, kind="ExternalOutput")
    ccout_d = [
        nc.dram_tensor(f"ccout{hl}", (128, EB, E), BF16, kind="Internal")
        for hl in range(2)
    ]
    ccred_d = [
        nc.dram_tensor(f"ccred{hl}", (128, EB, E), BF16, kind="Internal")
        for hl in range(2)
    ]

    with tile.TileContext(nc) as tc, ExitStack() as ctx:
        consts = ctx.enter_context(tc.tile_pool(name="consts", bufs=1))
        stream = ctx.enter_context(tc.tile_pool(name="stream", bufs=4))
        wqkv_pool = ctx.enter_context(tc.tile_pool(name="wqkv", bufs=2))
        t1_pool = ctx.enter_context(tc.tile_pool(name="t1", bufs=1))
        a_pool = ctx.enter_context(tc.tile_pool(name="a", bufs=2))
        p_pool = ctx.enter_context(tc.tile_pool(name="p", bufs=2))
        o_pool = ctx.enter_context(tc.tile_pool(name="o", bufs=4))
        r_pool = ctx.enter_context(tc.tile_pool(name="r", bufs=2))
        mm_ps = ctx.enter_context(tc.tile_pool(name="mmps", bufs=2, space=PSUM))
        big_ps = ctx.enter_context(tc.tile_pool(name="bigps", bufs=1, space=PSUM))
        u_ps = ctx.enter_context(tc.tile_pool(name="ups", bufs=2, space=PSUM))

        # PE warm-up: spin matmuls on a memset tile while the first DMAs are
        # in flight, so the tensor engine reaches its boosted p-state before
        # pass 1 starts (it needs ~3us of continuous work to clock up).
        warm_sb = consts.tile([128, 256], BF16, tag="warm")
        nc.gpsimd.memset(warm_sb[:], 0.0)
        warm_ps = mm_ps.tile([128, 256], F32, tag="mm", name="warm")
        for i in range(14):
            nc.tensor.matmul(
                warm_ps[:],
                warm_sb[:, 0:128],
                warm_sb[:],
                start=i == 0,
                stop=i == 13,
            )

        # x rows are pre-permuted host-side: chunks 0..7 are this core's own
        # n-half (used for the G contraction), chunks 8..15 the partner's
        # (X = x^T x is row-permutation invariant).  wp holds only the own
        # half.  wp1 interleaves with the odd xn chunks on the scalar queue
        # so every pass-1 chunk (xn[n], wp0[n], wp1[n]) lands just in time.
        NG = N // 256  # 8 G-contraction chunks
        xn_sb = [None] * NT
        wp1_sb = []
        for n in range(NG):
            t = consts.tile([128, E], BF16, tag=f"xn{n}", name=f"xn{n}")
            eng = nc.sync if n % 2 == 0 else nc.scalar
            eng.dma_start(out=t[:], in_=xn_d.ap()[n * 128 : (n + 1) * 128, :])
            xn_sb[n] = t
            if n % 2 == 1:
                w1 = stream.tile([128, E], BF16, tag="wp1", name=f"wp1_{n-1}", bufs=8)
                nc.scalar.dma_start(
                    out=w1[:], in_=wp_d.ap()[1, (n - 1) * 128 : n * 128, :]
                )
                wp1_sb.append(w1)
                w1 = stream.tile([128, E], BF16, tag="wp1", name=f"wp1_{n}", bufs=8)
                nc.scalar.dma_start(
                    out=w1[:], in_=wp_d.ap()[1, n * 128 : (n + 1) * 128, :]
                )
                wp1_sb.append(w1)

        # ---- other resident constants (scalar queue) ----
        biasx_sb = consts.tile([2, 2, 2, 4, E], BF16, tag="biasx")
        nc.scalar.dma_start(out=biasx_sb[:], in_=biasx_d.ap())
        eye_sb = consts.tile([128, 128], BF16, tag="eye")
        nc.scalar.dma_start(out=eye_sb[:], in_=eye_d.ap())
        onescol_sb = consts.tile([128, 1], BF16, tag="onescol")
        nc.scalar.dma_start(out=onescol_sb[:], in_=onescol_d.ap())

        # ---- pass 1: G0p + G1p = own-half x^T Wp halves (8 chunks) ----
        g_sb = [
            consts.tile([128, EB, E], BF16, tag=f"g{hl}", name=f"g{hl}")
            for hl in range(2)
        ]
        g0_slots = [
            mm_ps.tile([128, E], F32, tag="mm", name="g0a"),
            mm_ps.tile([128, E], F32, tag="mm", name="g0b"),
            u_ps.tile([128, E], F32, tag="u", name="g0c"),
            u_ps.tile([128, E], F32, tag="u", name="g0d"),
        ]
        g1_big = big_ps.tile([128, EB, E], F32, tag="big", name="g1big")
        gate_p1c4 = None
        for n in range(NG):
            nsl = slice(n * 128, (n + 1) * 128)
            wp0_sb = stream.tile([128, E], BF16, tag="wp0", name=f"wp0_{n}", bufs=8)
            nc.gpsimd.dma_start(out=wp0_sb[:], in_=wp_d.ap()[0, nsl, :])
            for m in range(EB):
                msl = slice(m * 128, (m + 1) * 128)
                g_bi = nc.tensor.matmul(
                    g0_slots[m][:],
                    xn_sb[n][:, msl],
                    wp0_sb[:],
                    start=n == 0,
                    stop=n == NG - 1,
                )
                nc.tensor.matmul(
                    g1_big[:, m, :],
                    xn_sb[n][:, msl],
                    wp1_sb[n][:],
                    start=n == 0,
                    stop=n == NG - 1,
                )
                if n == 4 and m == 0:
                    gate_p1c4 = g_bi.ins
        # partner's x half (for X only, needed from pass 2 on; execution
        # gated so it doesn't steal pass-1 bandwidth)
        for n in range(NG, NT):
            t = consts.tile([128, E], BF16, tag=f"xn{n}", name=f"xn{n}")
            eng = nc.sync if n % 2 == 0 else nc.scalar
            bi = eng.dma_start(out=t[:], in_=xn_d.ap()[n * 128 : (n + 1) * 128, :])
            tile.add_dep_helper(bi.ins, gate_p1c4, reason="delay partner x")
            xn_sb[n] = t
        # psum -> sbuf (partials), then pairwise AllReduce via DRAM, G0
        # first so the first head's P never waits.
        for m in range(EB):
            if m < 2:
                nc.scalar.activation(
                    out=g_sb[0][:, m, :],
                    in_=g0_slots[m][:],
                    func=mybir.ActivationFunctionType.Copy,
                )
                nc.vector.tensor_copy(g_sb[1][:, m, :], g1_big[:, m, :])
            else:
                nc.vector.tensor_copy(g_sb[0][:, m, :], g0_slots[m][:])
                nc.scalar.activation(
                    out=g_sb[1][:, m, :],
                    in_=g1_big[:, m, :],
                    func=mybir.ActivationFunctionType.Copy,
                )
        cc_groups = [[0, 1], [2, 3], [4, 5], [6, 7]]
        for hl in range(2):
            out_bi = nc.gpsimd.dma_start(out=ccout_d[hl].ap(), in_=g_sb[hl][:])
            cc = nc.gpsimd.collective_compute(
                kind="AllReduce",
                op=mybir.AluOpType.add,
                replica_groups=cc_groups,
                ins=[ccout_d[hl].ap()],
                outs=[ccred_d[hl].ap()],
            )
            tile.add_dep_helper(cc.ins, out_bi.ins, reason="cc waits dma out")
            in_bi = nc.gpsimd.dma_start(out=g_sb[hl][:], in_=ccred_d[hl].ap())
            tile.add_dep_helper(in_bi.ins, cc.ins, reason="dma in waits cc")

        # ---- pass 2: X = x^T x (upper block-triangle, all 16 chunks) ----
        X_sb = consts.tile([128, EB, E], BF16, tag="X")
        x_slots = [
            mm_ps.tile([128, E], F32, tag="mm", name="xm0"),
            mm_ps.tile([128, E], F32, tag="mm", name="xm1"),
            u_ps.tile([128, E], F32, tag="u", name="xm2"),
            u_ps.tile([128, E], F32, tag="u", name="xm3"),
        ]
        gate_p2start = None
        gate_gmid = None
        for n in range(NT):
            for m in range(EB):
                msl = slice(m * 128, (m + 1) * 128)
                x_bi = nc.tensor.matmul(
                    x_slots[m][:, 0 : E - m * 128],
                    xn_sb[n][:, msl],
                    xn_sb[n][:, m * 128 :],
                    start=n == 0,
                    stop=n == NT - 1,
                )
                if n == 2 and m == 0:
                    gate_p2start = x_bi.ins
                if n == NT // 2 and m == 0:
                    gate_gmid = x_bi.ins
        for m in range(EB):
            if m < 2:
                nc.scalar.activation(
                    out=X_sb[:, m, m * 128 :],
                    in_=x_slots[m][:, 0 : E - m * 128],
                    func=mybir.ActivationFunctionType.Copy,
                )
            else:
                nc.vector.tensor_copy(
                    X_sb[:, m, m * 128 :], x_slots[m][:, 0 : E - m * 128]
                )
        xtrans_ps = big_ps.tile([128, EB, E], BF16, tag="big", name="xtrans")
        for tm in range(1, EB):
            for tec in range(tm):
                tpo = xtrans_ps[:, tm, tec * 128 : (tec + 1) * 128]
                nc.tensor.transpose(
                    tpo, X_sb[:, tec, tm * 128 : (tm + 1) * 128], eye_sb[:]
                )
                nc.vector.tensor_copy(
                    X_sb[:, tm, tec * 128 : (tec + 1) * 128], tpo
                )

        gate_hist = [gate_p1c4, gate_p2start]  # per-head early gates
        del gate_gmid
        pending_s = None

        def emit_pending_s():
            nonlocal pending_s
            if pending_s is None:
                return
            A_sb, R_list = pending_s
            pending_s = None
            # s row = ones^T @ A  (A moving: only 4 cheap stationary loads)
            srow_ps = mm_ps.tile([1, E], F32, tag="mm", name="srow")
            for fc in range(EB):
                nc.tensor.matmul(
                    srow_ps[:],
                    onescol_sb[:],
                    A_sb[:, fc, :],
                    start=fc == 0,
                    stop=fc == EB - 1,
                )
            srow_sb = r_pool.tile([1, E], BF16, tag="srow")
            nc.vector.tensor_copy(srow_sb[:], srow_ps[:])
            # transpose 128-wide pieces into e-partition layout ([128, EB, 2]
            # keeps each bf16 column 4-byte aligned in PSUM)
            sT_ps = mm_ps.tile([128, EB, 2], BF16, tag="mm", name="sT")
            for eb in range(EB):
                nc.tensor.transpose(
                    sT_ps[:, eb, 0:1],
                    srow_sb[0:1, eb * 128 : (eb + 1) * 128],
                    eye_sb[0:1, 0:1],
                )
            r_sb = r_pool.tile([128, EB], F32, tag="r")
            nc.vector.reciprocal(out=r_sb[:], in_=sT_ps[:, :, 0])
            R_list.append(r_sb)

        for u in range(2):
            A_tiles, P_tiles, R_tiles = [], [], []
            for hl in range(2):
                # --- weights for head (u, hl), prefetch-gated (sync queue:
                # the gpsimd queue is owned by the collectives) ---
                wq_sb = wqkv_pool.tile([128, EB, E], BF16, tag="wq")
                wq_bi = nc.sync.dma_start(out=wq_sb[:], in_=wq_d.ap()[u, hl])
                wv_sb = wqkv_pool.tile([128, EB, E], BF16, tag="wv")
                wv_bi = nc.sync.dma_start(out=wv_sb[:], in_=wv_d.ap()[u, hl])
                wk_sb = wqkv_pool.tile([128, EB, E], BF16, tag="wk")
                wk_bi = nc.scalar.dma_start(out=wk_sb[:], in_=wk_d.ap()[u, hl])
                gate = gate_hist[-2]  # two head-phases back
                for bi in (wq_bi, wv_bi, wk_bi):
                    tile.add_dep_helper(bi.ins, gate, reason="delay prefetch")

                # --- T1 = X @ Wq_h, one m-block per PSUM bank; copies split
                # over ACT and DVE so attT can chase them block-by-block.
                # The previous head's exp runs on ACT during these matmuls.
                T1_sbs = []
                t1_first = None
                for m in range(EB):
                    msl = slice(m * 128, (m + 1) * 128)
                    t1_ps = mm_ps.tile([128, E], F32, tag="mm", name=f"t1p{m}")
                    for ec in range(EB):
                        bi = nc.tensor.matmul(
                            t1_ps[:],
                            X_sb[:, ec, msl],
                            wq_sb[:, ec, :],
                            start=ec == 0,
                            stop=ec == EB - 1,
                        )
                        t1_first = t1_first or bi
                    t1_sb = t1_pool.tile([128, E], BF16, tag=f"t1{m}")
                    if m < 2:
                        nc.scalar.activation(
                            out=t1_sb[:],
                            in_=t1_ps[:],
                            func=mybir.ActivationFunctionType.Copy,
                        )
                    else:
                        nc.vector.tensor_copy(t1_sb[:], t1_ps[:])
                    T1_sbs.append(t1_sb)
                gate_early = t1_first.ins

                # --- P_h = Wv_h^T @ G_hl + bv_h (x) swp_hl + bp/2 ---
                # (independent of T1/attT; covers the T1 copy latency)
                P_sb = p_pool.tile([128, EB, E], BF16, tag="p")

                def emit_p_group(fb):
                    fsl = slice(fb * 128, (fb + 1) * 128)
                    p_ps = u_ps.tile([128, E], F32, tag="u", name=f"pp{fb}")
                    for ec in range(EB):
                        nc.tensor.matmul(
                            p_ps[:],
                            wv_sb[:, ec, fsl],
                            g_sb[hl][:, ec, :],
                            start=ec == 0,
                            stop=False,
                        )
                    nc.tensor.matmul(
                        p_ps[:],
                        biasx_sb[0:2, u, hl, 2, fsl],
                        biasx_sb[0:2, u, hl, 3, :],
                        start=False,
                        stop=True,
                    )
                    nc.vector.tensor_copy(P_sb[:, fb, :], p_ps[:])

                emit_p_group(0)
                emit_p_group(1)

                # --- attT = Wk_h^T @ T1 + [hvec0;bk] (x) [bq;hvec1] ---
                attT_ps = big_ps.tile([128, EB, E], F32, tag="big")
                for fb in range(EB):
                    fsl = slice(fb * 128, (fb + 1) * 128)
                    for ec in range(EB):
                        nc.tensor.matmul(
                            attT_ps[:, fb, :],
                            wk_sb[:, ec, fsl],
                            T1_sbs[ec][:],
                            start=ec == 0,
                            stop=False,
                        )
                    nc.tensor.matmul(
                        attT_ps[:, fb, :],
                        biasx_sb[0:2, u, hl, 0, fsl],
                        biasx_sb[0:2, u, hl, 1, :],
                        start=False,
                        stop=True,
                    )

                # --- exp (softmax numerator, transposed layout) ---
                A_sb = a_pool.tile([128, EB, E], BF16, tag="a")
                for fb in range(EB):
                    nc.scalar.activation(
                        out=A_sb[:, fb, :],
                        in_=attT_ps[:, fb, :],
                        func=mybir.ActivationFunctionType.Exp,
                        scale=SCALE,
                    )

                # second half of P + previous head's s fill the PE while
                # exp runs on ACT
                emit_p_group(2)
                emit_p_group(3)
                P_tiles.append(P_sb)
                emit_pending_s()
                A_tiles.append(A_sb)
                pending_s = (A_sb, R_tiles)
                gate_hist.append(gate_early)

            # --- U_h = A_h^T @ P_h ; out = U0*r0 + U1*r1 ---
            out_tiles = [
                o_pool.tile([128, E], BF16, tag="o", name=f"ot{u}_{i}")
                for i in range(EB)
            ]
            for hl in range(2):
                for eb in range(EB):
                    if hl == 0 and eb == 2:
                        emit_pending_s()  # s of this unit's second head
                    esl = slice(eb * 128, (eb + 1) * 128)
                    u_tile = u_ps.tile([128, E], F32, tag="u")
                    for fc in range(EB):
                        nc.tensor.matmul(
                            u_tile[:],
                            A_tiles[hl][:, fc, esl],
                            P_tiles[hl][:, fc, :],
                            start=fc == 0,
                            stop=fc == EB - 1,
                        )
                    if hl == 0:
                        nc.vector.tensor_scalar_mul(
                            out_tiles[eb][:], u_tile[:], R_tiles[0][:, eb : eb + 1]
                        )
                    else:
                        nc.vector.scalar_tensor_tensor(
                            out_tiles[eb][:],
                            u_tile[:],
                            R_tiles[1][:, eb : eb + 1],
                            out_tiles[eb][:],
                            op0=mybir.AluOpType.mult,
                            op1=mybir.AluOpType.add,
                        )
                    if hl == 1:
                        nc.sync.dma_start(
                            out=out_d.ap()[u, eb * 128 : (eb + 1) * 128, :],
                            in_=out_tiles[eb][:],
                        )

    nc.compile()
    return nc


def _get_nc():
    global _CACHED_NC
    if _CACHED_NC is None:
        _CACHED_NC = build_nc()
    return _CACHED_NC


def make_in_maps(x, Wq, bq, Wk, bk, Wv, bv, Wp, bp):
    x = np.asarray(x, np.float32)
    Wq, Wk, Wv, Wp = (np.asarray(a, np.float32) for a in (Wq, Wk, Wv, Wp))
    bq, bk, bv, bp = (np.asarray(a, np.float32) for a in (bq, bk, bv, bp))
    swp = np.stack([Wp[:N].sum(0), Wp[N:].sum(0)])
    bph = 0.5 * bp
    in_maps = []
    for c in range(8):
        b = c // 2
        half = c % 2
        nh = N // 2
        hsl = slice(half * nh, (half + 1) * nh)
        osl = slice((1 - half) * nh, (2 - half) * nh)
        # own n-half first (G contraction reads chunks 0..7); X is
        # row-permutation invariant
        xr = np.concatenate([x[b][hsl], x[b][osl]])
        wp_arr = np.ascontiguousarray(
            np.stack([Wp[:N][hsl], Wp[N:][hsl]])
        ).astype(BF16_NP)
        rs = [2 * (c % 2), 2 * (c % 2) + 1]
        heads = [[2 * r + hl for hl in range(2)] for r in rs]
        xs = x[b].sum(0)

        def tile_w(Wm, h):
            # (E, E) -> [p, t, e] with row t*128+p on partition p
            return Wm[:, h::H].reshape(EB, 128, E).transpose(1, 0, 2)

        wq_arr = np.ascontiguousarray(
            np.stack([[tile_w(Wq, h) for h in hu] for hu in heads])
        ).astype(BF16_NP)
        wk_arr = np.ascontiguousarray(
            np.stack([[tile_w(Wk, h) for h in hu] for hu in heads])
        ).astype(BF16_NP)
        wv_arr = np.ascontiguousarray(
            np.stack([[tile_w(Wv, h) for h in hu] for hu in heads])
        ).astype(BF16_NP)
        biasx = np.zeros((2, 2, 2, 4, E), np.float32)
        for ui, hu in enumerate(heads):
            for hli, h in enumerate(hu):
                biasx[0, ui, hli, 0] = Wk[:, h::H].T @ xs
                biasx[1, ui, hli, 0] = bk[h::H]
                biasx[0, ui, hli, 1] = bq[h::H]
                biasx[1, ui, hli, 1] = Wq[:, h::H].T @ xs + np.float32(N) * bq[h::H]
                biasx[0, ui, hli, 2] = bv[h::H]
                biasx[1, ui, hli, 2] = 1.0
                biasx[0, ui, hli, 3] = swp[hli]
                biasx[1, ui, hli, 3] = bph
        in_maps.append(
            {
                "xn": np.ascontiguousarray(xr).astype(BF16_NP),
                "wq": wq_arr,
                "wk": wk_arr,
                "wv": wv_arr,
                "wp": wp_arr,
                "biasx": biasx.astype(BF16_NP),
                "eye": np.eye(128, dtype=BF16_NP),
                "onescol": np.ones((128, 1), BF16_NP),
            }
        )
    return in_maps


def assemble_out(results):
    out = np.empty((B, N, E), np.float32)
    for c in range(8):
        b = c // 2
        for ui in range(2):
            r = 2 * (c % 2) + ui
            out[b, r::4, :] = results[c]["out"][ui].astype(np.float32)
    return out


def run(inputs, trace=False, **spmd_kwargs):
    """Full pipeline; returns (output, BassKernelResults)."""
    nc = _get_nc()
    in_maps = make_in_maps(**inputs)
    res = run_bass_kernel_spmd(
        nc, in_maps, core_ids=list(range(8)), trace=trace, **spmd_kwargs
    )
    return assemble_out(res.results), res


def kernel(**inputs):
    out, _ = run(inputs)
    return out
